# revision 43
# baseline (speedup 1.0000x reference)
"""Distributed masked-attention kernel for one TRN2 chip (8 NeuronCores).

Problem: B=4, S=4096, IN=512, D=64 attention with a [S,S] int32 score mask
(masked scores replaced by 1e-6 *before* softmax, so masked probs are
exp(1e-6)/Z ~= 1/Z, NOT zero).

Sharding (8 cores): core c = b*2 + qh -> batch b in {0..3}, query rows
[2048*qh, 2048*(qh+1)). One batch per core halves the (replicated) KV
projection work vs 2-batch sharding - PE drops from ~95us to ~80us of work
and stops being the binding engine. Inputs are rolled along S so the core's
own query slab is at rows [0:2048) (attention's k-sum is permutation
invariant) -> all 8 cores run the IDENTICAL graph (SPMD).

The two pipeline streams are the q-halves qh' in {0,1} (1024 queries each),
playing the role the two batches played before:
  PE:  S^T = (K^T block)^T @ Q^T  (bf16, N=512 x2; Q^T zero-padded to 128
       partitions -> full-array contraction keeps the PE HAM at 2.4GHz)
  DVE: sm = S^T * mask -> SBUF bf16  (PSUM frees after the TT, not the exp)
  ACT: P = exp(0.125 * sm)           (masked -> exp(0) = 1 ~ ref's exp(1e-6))
  PE:  O^T[65, q] += V_aug^T @ P     (V_aug = [V | 1]: ones column emits the
                                      softmax denominator for free)
PV lags one k-tile so the in-order PE queue never head-of-line blocks.

DMA: x bf16 + mask int8 (host-cast; the kernel computed in bf16 anyway) =
13.2MB/core. Each HWDGE ring drains FIFO at ~150GB/s and DMA triggers park
sem-lane-reuse waits on the issuing engine's queue, so transfers are
split/ordered by first use across both rings with at most 5 triggers on the
scalar (ACT) queue (x in [128,1024] column-chunks, mask in 4-k-tile 1MB
groups) -> attention loop starts ~30us, runs 2.9us/k-tile.
"""

import sys

if "/opt/trn_rl_repo" not in sys.path:
    sys.path.insert(0, "/opt/trn_rl_repo")

from contextlib import ExitStack

import numpy as np

import concourse.bass as bass
import concourse.bacc as bacc
import concourse.mybir as mybir
import concourse.tile as tile
from concourse.bass_utils import run_bass_kernel_spmd
from concourse.masks import make_identity

ts = bass.ts
ds = bass.ds

N_CORES = 8
B, S, C, D = 4, 4096, 512, 64
Q_LOC = 2048       # query rows per core
QH = 1024          # one pipeline stream's q width
N_KT = S // 128    # 32 k-tiles of 128
QC = 512           # matmul moving chunk
N_MG = 8           # mask DMA groups (4 k-tiles each)

F32 = mybir.dt.float32
BF16 = mybir.dt.bfloat16
I8 = mybir.dt.int8
AF = mybir.ActivationFunctionType
ALU = mybir.AluOpType


def build_kernel() -> bacc.Bacc:
    nc = bacc.Bacc(None, target_bir_lowering=False, debug=False)

    xt_ext = nc.declare_dram_parameter("xt", [C, S], BF16, isOutput=False)
    mt_ext = nc.declare_dram_parameter("maskp", [N_MG, 128, 4 * Q_LOC], I8, isOutput=False)
    wkv_ext = nc.declare_dram_parameter("wkv", [128, 4, 2 * D], BF16, isOutput=False)
    wq_ext = nc.declare_dram_parameter("wq", [128, 4, D], BF16, isOutput=False)
    bkv_ext = nc.declare_dram_parameter("bkv", [2 * D, 1], F32, isOutput=False)
    bq_ext = nc.declare_dram_parameter("bq", [D, 1], F32, isOutput=False)
    out_ext = nc.declare_dram_parameter("out", [Q_LOC, D], F32, isOutput=True)

    with tile.TileContext(nc) as tc, ExitStack() as ctx:
        # ---------------- pools ----------------
        persist = ctx.enter_context(tc.tile_pool(name="persist", bufs=1))
        pt_pool = ctx.enter_context(tc.tile_pool(name="pt", bufs=8))
        sm_pool = ctx.enter_context(tc.tile_pool(name="sm", bufs=6))
        epi = ctx.enter_context(tc.tile_pool(name="epi", bufs=1))
        epi2 = ctx.enter_context(tc.tile_pool(name="epi2", bufs=2))
        psum_s = ctx.enter_context(
            tc.tile_pool(name="psum_s", bufs=2, space=bass.MemorySpace.PSUM)
        )
        psum_o = ctx.enter_context(
            tc.tile_pool(name="psum_o", bufs=1, space=bass.MemorySpace.PSUM)
        )

        # ---------------- upfront DMAs, ordered by first use ----------------
        wkv = persist.tile([128, 4, 2 * D], BF16)
        wq = persist.tile([128, 4, D], BF16)
        bias_kv = persist.tile([128, 1], F32)
        bias_q = persist.tile([D, 1], F32)
        xb = persist.tile([128, 4, S], BF16, name="xb", tag="xb")
        mg = [persist.tile([128, 4, Q_LOC], I8, name=f"mg{g}", tag=f"mg{g}") for g in range(N_MG)]
        kvt = persist.tile([128, S], BF16, name="kvt", tag="kvt")
        qt_t = persist.tile([128, Q_LOC], BF16, name="qt", tag="qt")
        vaug = persist.tile([128, N_KT, D + 1], BF16, name="va", tag="va")
        nc.gpsimd.memset(qt_t[D:128, :], 0.0)
        nc.gpsimd.memset(vaug[:, :, D : D + 1], 1.0)

        # The two HWDGE rings drain FIFO and share ~300GB/s of HBM, so the
        # critical-path transfers (x columns c0/c1 -> KV chunk 0 + both Q
        # projections; mask g0) lead their rings and the rest follows in
        # deadline order: SP ring w, x[c0], x[c2], x[c3], g4..g7; ACT ring
        # g0, x[c1], g1..g3.
        def dma_x(ring, cc):
            for j in range(4):
                ring.dma_start(xb[:, j, ts(cc, 1024)], xt_ext[ts(j, 128), ts(cc, 1024)])

        def dma_m(ring, g):
            ring.dma_start(mg[g][:].rearrange("p t q -> p (t q)"), mt_ext[g])

        # The scalar (ACT) queue gets only 5 DMA triggers (g0 + x[c1]) so no
        # DMA sem-lane-reuse wait can park on it ahead of the KV bias-copies;
        # everything else rides the SP queue, interleaved by deadline.
        nc.sync.dma_start(wkv[:], wkv_ext[:])
        nc.sync.dma_start(wq[:], wq_ext[:])
        nc.sync.dma_start(bias_kv[0 : 2 * D, :], bkv_ext[:])
        nc.sync.dma_start(bias_q[:], bq_ext[:])
        dma_x(nc.scalar, 0)
        dma_m(nc.scalar, 0)
        dma_x(nc.sync, 1)
        dma_m(nc.sync, 1)
        dma_x(nc.sync, 2)
        dma_m(nc.sync, 2)
        dma_m(nc.sync, 3)
        dma_x(nc.sync, 3)
        for g in range(4, N_MG):
            dma_m(nc.sync, g)

        # ---------------- constants ----------------
        ident_f = persist.tile([128, 128], F32)
        make_identity(nc, ident_f[:])
        ident_b = persist.tile([128, 128], BF16)
        make_identity(nc, ident_b[:])

        def emit_kv_half(c: int, h: int):
            kv_ps = psum_s.tile([128, QC], F32, name="kvps", tag="ps")
            for j in range(4):
                nc.tensor.matmul(
                    kv_ps[:],
                    wkv[:, j, :],
                    xb[:, j, ds(c * 1024 + h * QC, QC)],
                    start=(j == 0),
                    stop=(j == 3),
                )
            nc.scalar.activation(
                kvt[:, ds(c * 1024 + h * QC, QC)], kv_ps[:], AF.Identity,
                bias=bias_kv[:],
            )
            vp = psum_s.tile([128, 4, D], BF16, name="vp", tag="ps")
            kt0 = 8 * c + 4 * h
            for u in range(4):
                nc.tensor.transpose(
                    vp[:, u, :],
                    kvt[D : 2 * D, ts(kt0 + u, 128)],
                    ident_b[D : 2 * D, D : 2 * D],
                )
            # ACT (not DVE): vp shares the rotating PSUM pool with the score
            # tiles, and the DVE queue is 2 mask-TTs deep - reading vp out on
            # the scalar engine frees the slab ~1.5us sooner per kv slot.
            nc.scalar.copy(vaug[:, kt0 : kt0 + 4, 0:D], vp[:])

        def emit_q(qh: int):
            q_ps = psum_s.tile([D, QH], F32, name="qps", tag="ps")
            for h in range(QH // QC):
                for j in range(4):
                    nc.tensor.matmul(
                        q_ps[:, ts(h, QC)],
                        wq[:, j, :],
                        xb[:, j, ds(qh * QH + h * QC, QC)],
                        start=(j == 0),
                        stop=(j == 3),
                    )
            nc.scalar.activation(
                qt_t[0:D, ts(qh, QH)], q_ps[:], AF.Identity, bias=bias_q[:]
            )

        def emit_scores_exp(qh, kt, mk):
            st = psum_s.tile([128, QH], F32, name="st", tag="ps")
            for qc in range(QH // QC):
                nc.tensor.matmul(
                    st[:, ts(qc, QC)],
                    kvt[:, ts(kt, 128)],
                    qt_t[:, ds(qh * QH + qc * QC, QC)],
                    start=True,
                    stop=True,
                )
            sm = sm_pool.tile([128, QH], BF16, tag="sm")
            nc.vector.tensor_tensor(out=sm[:], in0=st[:], in1=mk, op=ALU.mult)
            pt = pt_pool.tile([128, QH], BF16, tag="pt")
            nc.scalar.activation(pt[:], sm[:], AF.Exp, scale=0.125)
            return pt

        def emit_pv(qh, kt, ot, pt, first, last):
            for qc in range(QH // QC):
                nc.tensor.matmul(
                    ot[:, ds(qh * QH + qc * QC, QC)],
                    vaug[:, kt, :],
                    pt[:, ts(qc, QC)],
                    start=first,
                    stop=last,
                )

        def emit_epilogue_half(ot, half):
            # one q-half: PSUM->SBUF copy, 8 PE transposes, strided
            # reciprocal of the denominator row, divide, store
            ots = epi.tile([D + 1, QH], F32, name=f"ots{half}", tag=f"ots{half}")
            nc.scalar.copy(ots[:], ot[:, ts(half, QH)])
            rcp = epi2.tile([128, 16], F32, tag="rcp")
            of = epi2.tile([128, 8, D], F32, tag=f"of{half}")
            oext = out_ext[:].rearrange("(hf qt p) d -> hf p qt d", hf=2, p=128)
            op8 = psum_s.tile([128, 8, 128], F32, name="op8", tag="ps")
            for i in range(8):
                nc.tensor.transpose(
                    op8[:, i, 0 : D + 1], ots[:, ts(i, 128)],
                    ident_f[0 : D + 1, 0 : D + 1],
                )
            nc.vector.reciprocal(
                rcp[:, ts(half, 8)], op8[:, :, D : D + 1].rearrange("p t o -> p (t o)")
            )
            for i in range(8):
                nc.vector.tensor_scalar(
                    of[:, i, :], op8[:, i, 0:D], rcp[:, 8 * half + i : 8 * half + i + 1],
                    None, op0=ALU.mult,
                )
            nc.sync.dma_start(oext[half], of[:])

        # ---------------- emission ----------------
        ot = psum_o.tile([D + 1, Q_LOC], F32, name="ot", tag="ot")
        emit_kv_half(0, 0)
        emit_kv_half(0, 1)
        emit_q(0)
        emit_q(1)
        # KV chunk c+1 interleaves during chunk c's k-tiles (2 halves over
        # 8 k-tiles; x column-chunks stream in well ahead of their deadline)
        # PV lags TWO k-tiles behind the scores so the in-order PE queue
        # never waits on a just-finished exp.
        pending = []  # [(kt, [pt_qh0, pt_qh1]), ...]
        for c in range(4):
            nxt = []
            if c + 1 < 4:
                nxt = [(c + 1, h) for h in range(2)]
            for i, kt in enumerate(range(8 * c, 8 * c + 8)):
                # kv-half BEFORE this kt's scores: its matmuls fill the PE's
                # TT-wait gap and its ACT bias-copy precedes the exps in the
                # scalar queue, freeing the shared PSUM slab sooner
                if i in (2, 5) and nxt:
                    emit_kv_half(*nxt.pop(0))
                pts = []
                for qh in range(2):
                    mk = mg[kt // 4][:, kt % 4, ts(qh, QH)]
                    pts.append(emit_scores_exp(qh, kt, mk))
                pending.append((kt, pts))
                if len(pending) > 2:
                    pkt, ppts = pending.pop(0)
                    for qh in range(2):
                        emit_pv(qh, pkt, ot, ppts[qh], pkt == 0, False)
        # flush: the qh0 half of the epilogue overlaps the final qh1 PVs
        (k30, p30), (k31, p31) = pending
        for qh in range(2):
            emit_pv(qh, k30, ot, p30[qh], False, False)
        emit_pv(0, k31, ot, p31[0], False, True)
        emit_epilogue_half(ot, 0)
        emit_pv(1, k31, ot, p31[1], False, True)
        emit_epilogue_half(ot, 1)

    nc.compile()
    return nc


def _shard_inputs(input_embedding, mask, Wq, bq, Wk, bk, Wv, bv):
    import ml_dtypes

    input_embedding = np.asarray(input_embedding, dtype=np.float32)
    mask = np.asarray(mask, dtype=np.int32)

    def pack_w(w):
        return np.ascontiguousarray(
            np.asarray(w, np.float32).reshape(4, 128, -1).transpose(1, 0, 2)
        ).astype(ml_dtypes.bfloat16)

    wkv = np.concatenate([pack_w(Wk), pack_w(Wv)], axis=2)
    w = {
        "wkv": np.ascontiguousarray(wkv),
        "wq": pack_w(Wq),
        "bkv": np.ascontiguousarray(
            np.concatenate([np.asarray(bk, np.float32), np.asarray(bv, np.float32)]).reshape(-1, 1)
        ),
        "bq": np.ascontiguousarray(np.asarray(bq, np.float32).reshape(-1, 1)),
    }
    in_maps = []
    for c in range(N_CORES):
        b, qh = divmod(c, 2)
        # x^T [C, S] bf16, rolled so this core's q-slab is at [0:Q_LOC)
        x_c = np.roll(input_embedding[b].T, -Q_LOC * qh, axis=1).astype(
            ml_dtypes.bfloat16
        )
        # mask^T slab [S(k), Q_LOC(q)] rolled along k, packed so group g's
        # partition p holds k-rows {g*512 + t*128 + p} (8KB contiguous)
        m_c = np.roll(mask[Q_LOC * qh : Q_LOC * (qh + 1), :].T, -Q_LOC * qh, axis=0)
        m_p = (
            m_c.astype(np.int8)
            .reshape(N_MG, 4, 128, Q_LOC)
            .transpose(0, 2, 1, 3)
            .reshape(N_MG, 128, 4 * Q_LOC)
        )
        in_maps.append(
            {
                "xt": np.ascontiguousarray(x_c),
                "maskp": np.ascontiguousarray(m_p),
                **w,
            }
        )
    return in_maps


def _gather(results):
    out = np.empty((B, S, D), dtype=np.float32)
    for c in range(N_CORES):
        b, qh = divmod(c, 2)
        out[b, Q_LOC * qh : Q_LOC * (qh + 1), :] = results[c]["out"]
    return out


def kernel(input_embedding, mask, Wq, bq, Wk, bk, Wv, bv):
    nc = build_kernel()
    in_maps = _shard_inputs(input_embedding, mask, Wq, bq, Wk, bk, Wv, bv)
    res = run_bass_kernel_spmd(nc, in_maps, list(range(N_CORES)))
    return _gather(res.results)


# revision 44
# speedup vs baseline: 1.1751x; 1.1751x over previous
"""Distributed masked-attention kernel for one TRN2 chip (8 NeuronCores).

Problem: B=4, S=4096, IN=512, D=64 attention with a [S,S] int32 score mask
(masked scores replaced by 1e-6 *before* softmax, so masked probs are
exp(1e-6)/Z ~= 1/Z, NOT zero).

Sharding (8 cores): core c = b*2 + qh -> batch b in {0..3}, query rows
[2048*qh, 2048*(qh+1)). One batch per core halves the (replicated) KV
projection work vs 2-batch sharding - PE drops from ~95us to ~80us of work
and stops being the binding engine. Inputs are rolled along S so the core's
own query slab is at rows [0:2048) (attention's k-sum is permutation
invariant) -> all 8 cores run the IDENTICAL graph (SPMD).

The two pipeline streams are the q-halves qh' in {0,1} (1024 queries each),
playing the role the two batches played before:
  PE:  S^T = (K^T block)^T @ Q^T  (bf16, N=512 x2; Q^T zero-padded to 128
       partitions -> full-array contraction keeps the PE HAM at 2.4GHz)
  DVE: sm = S^T * mask -> SBUF bf16  (PSUM frees after the TT, not the exp)
  ACT: P = exp(0.125 * sm)           (masked -> exp(0) = 1 ~ ref's exp(1e-6))
  PE:  O^T[65, q] += V_aug^T @ P     (V_aug = [V | 1]: ones column emits the
                                      softmax denominator for free)
PV lags one k-tile so the in-order PE queue never head-of-line blocks.

DMA: x bf16 + mask int8 (host-cast; the kernel computed in bf16 anyway) =
13.2MB/core. Each HWDGE ring drains FIFO at ~150GB/s and DMA triggers park
sem-lane-reuse waits on the issuing engine's queue, so transfers are
split/ordered by first use across both rings with at most 5 triggers on the
scalar (ACT) queue (x in [128,1024] column-chunks, mask in 4-k-tile 1MB
groups) -> attention loop starts ~30us, runs 2.9us/k-tile.
"""

import sys

if "/opt/trn_rl_repo" not in sys.path:
    sys.path.insert(0, "/opt/trn_rl_repo")

from contextlib import ExitStack

import numpy as np

import concourse.bass as bass
import concourse.bacc as bacc
import concourse.mybir as mybir
import concourse.tile as tile
from concourse.bass_utils import run_bass_kernel_spmd
from concourse.masks import make_identity

ts = bass.ts
ds = bass.ds

N_CORES = 8
B, S, C, D = 4, 4096, 512, 64
Q_LOC = 2048       # query rows per core
QH = 1024          # one pipeline stream's q width
N_KT = S // 128    # 32 k-tiles of 128
QC = 512           # matmul moving chunk
N_MG = 8           # mask DMA groups (4 k-tiles each)

F32 = mybir.dt.float32
BF16 = mybir.dt.bfloat16
I8 = mybir.dt.int8
AF = mybir.ActivationFunctionType
ALU = mybir.AluOpType


def build_kernel() -> bacc.Bacc:
    nc = bacc.Bacc(None, target_bir_lowering=False, debug=False)

    xt_ext = nc.declare_dram_parameter("xt", [C, S], BF16, isOutput=False)
    mt_ext = nc.declare_dram_parameter("maskp", [N_MG, 128, 4 * Q_LOC], I8, isOutput=False)
    wkv_ext = nc.declare_dram_parameter("wkv", [128, 4, 2 * D], BF16, isOutput=False)
    wq_ext = nc.declare_dram_parameter("wq", [128, 4, D], BF16, isOutput=False)
    bkv_ext = nc.declare_dram_parameter("bkv", [2 * D, 1], F32, isOutput=False)
    bq_ext = nc.declare_dram_parameter("bq", [D, 1], F32, isOutput=False)
    out_ext = nc.declare_dram_parameter("out", [Q_LOC, D], F32, isOutput=True)

    with tile.TileContext(nc) as tc, ExitStack() as ctx:
        # ---------------- pools ----------------
        persist = ctx.enter_context(tc.tile_pool(name="persist", bufs=1))
        pt_pool = ctx.enter_context(tc.tile_pool(name="pt", bufs=10))
        sm_pool = ctx.enter_context(tc.tile_pool(name="sm", bufs=8))
        epi = ctx.enter_context(tc.tile_pool(name="epi", bufs=1))
        epi2 = ctx.enter_context(tc.tile_pool(name="epi2", bufs=2))
        psum_s = ctx.enter_context(
            tc.tile_pool(name="psum_s", bufs=2, space=bass.MemorySpace.PSUM)
        )
        psum_o = ctx.enter_context(
            tc.tile_pool(name="psum_o", bufs=1, space=bass.MemorySpace.PSUM)
        )

        # ---------------- upfront DMAs, ordered by first use ----------------
        wkv = persist.tile([128, 4, 2 * D], BF16)
        wq = persist.tile([128, 4, D], BF16)
        bias_kv = persist.tile([128, 1], F32)
        bias_q = persist.tile([D, 1], F32)
        xb = persist.tile([128, 4, S], BF16, name="xb", tag="xb")
        mg = [persist.tile([128, 4, Q_LOC], I8, name=f"mg{g}", tag=f"mg{g}") for g in range(N_MG)]
        kvt = persist.tile([128, S], BF16, name="kvt", tag="kvt")
        qt_t = persist.tile([128, Q_LOC], BF16, name="qt", tag="qt")
        vaug = persist.tile([128, N_KT, D + 1], BF16, name="va", tag="va")
        nc.gpsimd.memset(qt_t[D:128, :], 0.0)
        nc.gpsimd.memset(vaug[:, :, D : D + 1], 1.0)

        # The two HWDGE rings drain FIFO and share ~300GB/s of HBM, so the
        # critical-path transfers (x columns c0/c1 -> KV chunk 0 + both Q
        # projections; mask g0) lead their rings and the rest follows in
        # deadline order: SP ring w, x[c0], x[c2], x[c3], g4..g7; ACT ring
        # g0, x[c1], g1..g3.
        def dma_x(ring, cc):
            for j in range(4):
                ring.dma_start(xb[:, j, ts(cc, 1024)], xt_ext[ts(j, 128), ts(cc, 1024)])

        def dma_m(ring, g):
            ring.dma_start(mg[g][:].rearrange("p t q -> p (t q)"), mt_ext[g])

        # The scalar (ACT) queue gets only 5 DMA triggers (g0 + x[c1]) so no
        # DMA sem-lane-reuse wait can park on it ahead of the KV bias-copies;
        # everything else rides the SP queue, interleaved by deadline.
        nc.sync.dma_start(wkv[:], wkv_ext[:])
        nc.sync.dma_start(wq[:], wq_ext[:])
        nc.sync.dma_start(bias_kv[0 : 2 * D, :], bkv_ext[:])
        nc.sync.dma_start(bias_q[:], bq_ext[:])
        dma_x(nc.scalar, 0)
        dma_m(nc.scalar, 0)
        dma_x(nc.sync, 1)
        dma_m(nc.sync, 1)
        dma_x(nc.sync, 2)
        dma_m(nc.sync, 2)
        dma_m(nc.sync, 3)
        dma_x(nc.sync, 3)
        for g in range(4, N_MG):
            dma_m(nc.sync, g)

        # ---------------- constants ----------------
        ident_f = persist.tile([128, 128], F32)
        make_identity(nc, ident_f[:])
        ident_b = persist.tile([128, 128], BF16)
        make_identity(nc, ident_b[:])

        def emit_kv_half(c: int, h: int):
            kv_ps = psum_s.tile([128, QC], F32, name="kvps", tag="ps")
            for j in range(4):
                nc.tensor.matmul(
                    kv_ps[:],
                    wkv[:, j, :],
                    xb[:, j, ds(c * 1024 + h * QC, QC)],
                    start=(j == 0),
                    stop=(j == 3),
                )
            nc.scalar.activation(
                kvt[:, ds(c * 1024 + h * QC, QC)], kv_ps[:], AF.Identity,
                bias=bias_kv[:],
            )
            vp = psum_s.tile([128, 4, D], BF16, name="vp", tag="ps")
            kt0 = 8 * c + 4 * h
            for u in range(4):
                nc.tensor.transpose(
                    vp[:, u, :],
                    kvt[D : 2 * D, ts(kt0 + u, 128)],
                    ident_b[D : 2 * D, D : 2 * D],
                )
            # ACT (not DVE): vp shares the rotating PSUM pool with the score
            # tiles, and the DVE queue is 2 mask-TTs deep - reading vp out on
            # the scalar engine frees the slab ~1.5us sooner per kv slot.
            nc.scalar.copy(vaug[:, kt0 : kt0 + 4, 0:D], vp[:])

        def emit_q(qh: int):
            q_ps = psum_s.tile([D, QH], F32, name="qps", tag="ps")
            for h in range(QH // QC):
                for j in range(4):
                    nc.tensor.matmul(
                        q_ps[:, ts(h, QC)],
                        wq[:, j, :],
                        xb[:, j, ds(qh * QH + h * QC, QC)],
                        start=(j == 0),
                        stop=(j == 3),
                    )
            nc.scalar.activation(
                qt_t[0:D, ts(qh, QH)], q_ps[:], AF.Identity, bias=bias_q[:]
            )

        def emit_scores_exp(qh, kt, mk):
            st = psum_s.tile([128, QH], F32, name="st", tag="ps")
            for qc in range(QH // QC):
                nc.tensor.matmul(
                    st[:, ts(qc, QC)],
                    kvt[:, ts(kt, 128)],
                    qt_t[:, ds(qh * QH + qc * QC, QC)],
                    start=True,
                    stop=True,
                )
            sm = sm_pool.tile([128, QH], BF16, tag="sm")
            nc.vector.tensor_tensor(out=sm[:], in0=st[:], in1=mk, op=ALU.mult)
            pt = pt_pool.tile([128, QH], BF16, tag="pt")
            nc.scalar.activation(pt[:], sm[:], AF.Exp, scale=0.125)
            return pt

        def emit_pv(qh, kt, ot, pt, first, last):
            for qc in range(QH // QC):
                nc.tensor.matmul(
                    ot[:, ds(qh * QH + qc * QC, QC)],
                    vaug[:, kt, :],
                    pt[:, ts(qc, QC)],
                    start=first,
                    stop=last,
                )

        def emit_epilogue_half(ot, half):
            # one q-half: PSUM->SBUF copy, 8 PE transposes, strided
            # reciprocal of the denominator row, divide, store
            ots = epi.tile([D + 1, QH], F32, name=f"ots{half}", tag=f"ots{half}")
            nc.scalar.copy(ots[:], ot[:, ts(half, QH)])
            rcp = epi2.tile([128, 16], F32, tag="rcp")
            of = epi2.tile([128, 8, D], F32, tag=f"of{half}")
            oext = out_ext[:].rearrange("(hf qt p) d -> hf p qt d", hf=2, p=128)
            op8 = psum_s.tile([128, 8, 128], F32, name="op8", tag="ps")
            for i in range(8):
                nc.tensor.transpose(
                    op8[:, i, 0 : D + 1], ots[:, ts(i, 128)],
                    ident_f[0 : D + 1, 0 : D + 1],
                )
            nc.vector.reciprocal(
                rcp[:, ts(half, 8)], op8[:, :, D : D + 1].rearrange("p t o -> p (t o)")
            )
            for i in range(8):
                nc.vector.tensor_scalar(
                    of[:, i, :], op8[:, i, 0:D], rcp[:, 8 * half + i : 8 * half + i + 1],
                    None, op0=ALU.mult,
                )
            nc.sync.dma_start(oext[half], of[:])

        # ---------------- emission ----------------
        ot = psum_o.tile([D + 1, Q_LOC], F32, name="ot", tag="ot")
        emit_kv_half(0, 0)
        emit_kv_half(0, 1)
        emit_q(0)
        emit_q(1)
        # KV chunk c+1 interleaves during chunk c's k-tiles (2 halves over
        # 8 k-tiles; x column-chunks stream in well ahead of their deadline)
        # PV lags TWO k-tiles behind the scores so the in-order PE queue
        # never waits on a just-finished exp.
        pending = []  # [(kt, [pt_qh0, pt_qh1]), ...]
        for c in range(4):
            nxt = []
            if c + 1 < 4:
                nxt = [(c + 1, h) for h in range(2)]
            for i, kt in enumerate(range(8 * c, 8 * c + 8)):
                # kv-half BEFORE this kt's scores: its matmuls fill the PE's
                # TT-wait gap and its ACT bias-copy precedes the exps in the
                # scalar queue, freeing the shared PSUM slab sooner
                if i in (2, 5) and nxt:
                    emit_kv_half(*nxt.pop(0))
                pts = []
                for qh in range(2):
                    mk = mg[kt // 4][:, kt % 4, ts(qh, QH)]
                    pts.append(emit_scores_exp(qh, kt, mk))
                pending.append((kt, pts))
                if len(pending) > 3:
                    pkt, ppts = pending.pop(0)
                    for qh in range(2):
                        emit_pv(qh, pkt, ot, ppts[qh], pkt == 0, False)
        # flush: the qh0 half of the epilogue overlaps the final qh1 PVs
        for pkt, ppts in pending[:-1]:
            for qh in range(2):
                emit_pv(qh, pkt, ot, ppts[qh], False, False)
        k31, p31 = pending[-1]
        emit_pv(0, k31, ot, p31[0], False, True)
        emit_epilogue_half(ot, 0)
        emit_pv(1, k31, ot, p31[1], False, True)
        emit_epilogue_half(ot, 1)

    nc.compile()
    return nc


def _shard_inputs(input_embedding, mask, Wq, bq, Wk, bk, Wv, bv):
    import ml_dtypes

    input_embedding = np.asarray(input_embedding, dtype=np.float32)
    mask = np.asarray(mask, dtype=np.int32)

    def pack_w(w):
        return np.ascontiguousarray(
            np.asarray(w, np.float32).reshape(4, 128, -1).transpose(1, 0, 2)
        ).astype(ml_dtypes.bfloat16)

    wkv = np.concatenate([pack_w(Wk), pack_w(Wv)], axis=2)
    w = {
        "wkv": np.ascontiguousarray(wkv),
        "wq": pack_w(Wq),
        "bkv": np.ascontiguousarray(
            np.concatenate([np.asarray(bk, np.float32), np.asarray(bv, np.float32)]).reshape(-1, 1)
        ),
        "bq": np.ascontiguousarray(np.asarray(bq, np.float32).reshape(-1, 1)),
    }
    in_maps = []
    for c in range(N_CORES):
        b, qh = divmod(c, 2)
        # x^T [C, S] bf16, rolled so this core's q-slab is at [0:Q_LOC)
        x_c = np.roll(input_embedding[b].T, -Q_LOC * qh, axis=1).astype(
            ml_dtypes.bfloat16
        )
        # mask^T slab [S(k), Q_LOC(q)] rolled along k, packed so group g's
        # partition p holds k-rows {g*512 + t*128 + p} (8KB contiguous)
        m_c = np.roll(mask[Q_LOC * qh : Q_LOC * (qh + 1), :].T, -Q_LOC * qh, axis=0)
        m_p = (
            m_c.astype(np.int8)
            .reshape(N_MG, 4, 128, Q_LOC)
            .transpose(0, 2, 1, 3)
            .reshape(N_MG, 128, 4 * Q_LOC)
        )
        in_maps.append(
            {
                "xt": np.ascontiguousarray(x_c),
                "maskp": np.ascontiguousarray(m_p),
                **w,
            }
        )
    return in_maps


def _gather(results):
    out = np.empty((B, S, D), dtype=np.float32)
    for c in range(N_CORES):
        b, qh = divmod(c, 2)
        out[b, Q_LOC * qh : Q_LOC * (qh + 1), :] = results[c]["out"]
    return out


def kernel(input_embedding, mask, Wq, bq, Wk, bk, Wv, bv):
    nc = build_kernel()
    in_maps = _shard_inputs(input_embedding, mask, Wq, bq, Wk, bk, Wv, bv)
    res = run_bass_kernel_spmd(nc, in_maps, list(range(N_CORES)))
    return _gather(res.results)


# revision 45
# speedup vs baseline: 1.1800x; 1.0041x over previous
"""Distributed masked-attention kernel for one TRN2 chip (8 NeuronCores).

Problem: B=4, S=4096, IN=512, D=64 attention with a [S,S] int32 score mask
(masked scores replaced by 1e-6 *before* softmax, so masked probs are
exp(1e-6)/Z ~= 1/Z, NOT zero).

Sharding (8 cores): core c = b*2 + qh -> batch b in {0..3}, query rows
[2048*qh, 2048*(qh+1)). One batch per core halves the (replicated) KV
projection work vs 2-batch sharding - PE drops from ~95us to ~80us of work
and stops being the binding engine. Inputs are rolled along S so the core's
own query slab is at rows [0:2048) (attention's k-sum is permutation
invariant) -> all 8 cores run the IDENTICAL graph (SPMD).

The two pipeline streams are the q-halves qh' in {0,1} (1024 queries each),
playing the role the two batches played before:
  PE:  S^T = (K^T block)^T @ Q^T  (bf16, N=512 x2; Q^T zero-padded to 128
       partitions -> full-array contraction keeps the PE HAM at 2.4GHz)
  DVE: sm = S^T * mask -> SBUF bf16  (PSUM frees after the TT, not the exp)
  ACT: P = exp(0.125 * sm)           (masked -> exp(0) = 1 ~ ref's exp(1e-6))
  PE:  O^T[65, q] += V_aug^T @ P     (V_aug = [V | 1]: ones column emits the
                                      softmax denominator for free)
PV lags one k-tile so the in-order PE queue never head-of-line blocks.

DMA: x bf16 + mask int8 (host-cast; the kernel computed in bf16 anyway) =
13.2MB/core. Each HWDGE ring drains FIFO at ~150GB/s and DMA triggers park
sem-lane-reuse waits on the issuing engine's queue, so transfers are
split/ordered by first use across both rings with at most 5 triggers on the
scalar (ACT) queue (x in [128,1024] column-chunks, mask in 4-k-tile 1MB
groups) -> attention loop starts ~30us, runs 2.9us/k-tile.
"""

import sys

if "/opt/trn_rl_repo" not in sys.path:
    sys.path.insert(0, "/opt/trn_rl_repo")

from contextlib import ExitStack

import numpy as np

import concourse.bass as bass
import concourse.bacc as bacc
import concourse.mybir as mybir
import concourse.tile as tile
from concourse.bass_utils import run_bass_kernel_spmd
from concourse.masks import make_identity

ts = bass.ts
ds = bass.ds

N_CORES = 8
B, S, C, D = 4, 4096, 512, 64
Q_LOC = 2048       # query rows per core
QH = 1024          # one pipeline stream's q width
N_KT = S // 128    # 32 k-tiles of 128
QC = 512           # matmul moving chunk
N_MG = 8           # mask DMA groups (4 k-tiles each)

F32 = mybir.dt.float32
BF16 = mybir.dt.bfloat16
I8 = mybir.dt.int8
AF = mybir.ActivationFunctionType
ALU = mybir.AluOpType


def build_kernel() -> bacc.Bacc:
    nc = bacc.Bacc(None, target_bir_lowering=False, debug=False)

    xt_ext = nc.declare_dram_parameter("xt", [C, S], BF16, isOutput=False)
    mt_ext = nc.declare_dram_parameter("maskp", [N_MG, 128, 4 * Q_LOC], I8, isOutput=False)
    wkv_ext = nc.declare_dram_parameter("wkv", [128, 4, 2 * D], BF16, isOutput=False)
    wq_ext = nc.declare_dram_parameter("wq", [128, 4, D], BF16, isOutput=False)
    bkv_ext = nc.declare_dram_parameter("bkv", [2 * D, 1], F32, isOutput=False)
    bq_ext = nc.declare_dram_parameter("bq", [D, 1], F32, isOutput=False)
    out_ext = nc.declare_dram_parameter("out", [Q_LOC, D], F32, isOutput=True)

    with tile.TileContext(nc) as tc, ExitStack() as ctx:
        # ---------------- pools ----------------
        persist = ctx.enter_context(tc.tile_pool(name="persist", bufs=1))
        pt_pool = ctx.enter_context(tc.tile_pool(name="pt", bufs=10))
        sm_pool = ctx.enter_context(tc.tile_pool(name="sm", bufs=8))
        epi = ctx.enter_context(tc.tile_pool(name="epi", bufs=1))
        epi2 = ctx.enter_context(tc.tile_pool(name="epi2", bufs=2))
        psum_s = ctx.enter_context(
            tc.tile_pool(name="psum_s", bufs=2, space=bass.MemorySpace.PSUM)
        )
        psum_o = ctx.enter_context(
            tc.tile_pool(name="psum_o", bufs=1, space=bass.MemorySpace.PSUM)
        )

        # ---------------- upfront DMAs, ordered by first use ----------------
        wkv = persist.tile([128, 4, 2 * D], BF16)
        wq = persist.tile([128, 4, D], BF16)
        bias_kv = persist.tile([128, 1], F32)
        bias_q = persist.tile([D, 1], F32)
        xb = persist.tile([128, 4, S], BF16, name="xb", tag="xb")
        mg = [persist.tile([128, 4, Q_LOC], I8, name=f"mg{g}", tag=f"mg{g}") for g in range(N_MG)]
        kvt = persist.tile([128, S], BF16, name="kvt", tag="kvt")
        qt_t = persist.tile([128, Q_LOC], BF16, name="qt", tag="qt")
        vaug = persist.tile([128, N_KT, D + 1], BF16, name="va", tag="va")
        nc.gpsimd.memset(qt_t[D:128, :], 0.0)
        nc.gpsimd.memset(vaug[:, :, D : D + 1], 1.0)

        # The two HWDGE rings drain FIFO and share ~300GB/s of HBM, so the
        # critical-path transfers (x columns c0/c1 -> KV chunk 0 + both Q
        # projections; mask g0) lead their rings and the rest follows in
        # deadline order: SP ring w, x[c0], x[c2], x[c3], g4..g7; ACT ring
        # g0, x[c1], g1..g3.
        def dma_x(ring, cc):
            for j in range(4):
                ring.dma_start(xb[:, j, ts(cc, 1024)], xt_ext[ts(j, 128), ts(cc, 1024)])

        def dma_m(ring, g):
            ring.dma_start(mg[g][:].rearrange("p t q -> p (t q)"), mt_ext[g])

        # The scalar (ACT) queue gets only 5 DMA triggers (g0 + x[c1]) so no
        # DMA sem-lane-reuse wait can park on it ahead of the KV bias-copies;
        # everything else rides the SP queue, interleaved by deadline.
        # g0 leads the SYNC ring (lands ~7.5us) while x[c0] has the scalar
        # ring to itself - the first TT no longer waits for the mask.
        dma_m(nc.sync, 0)
        nc.sync.dma_start(wkv[:], wkv_ext[:])
        nc.sync.dma_start(wq[:], wq_ext[:])
        nc.sync.dma_start(bias_kv[0 : 2 * D, :], bkv_ext[:])
        nc.sync.dma_start(bias_q[:], bq_ext[:])
        dma_x(nc.scalar, 0)
        dma_x(nc.sync, 1)
        dma_m(nc.sync, 1)
        dma_m(nc.sync, 2)
        dma_x(nc.sync, 2)
        dma_m(nc.sync, 3)
        dma_m(nc.sync, 4)
        dma_x(nc.sync, 3)
        for g in range(5, N_MG):
            dma_m(nc.sync, g)

        # ---------------- constants ----------------
        ident_f = persist.tile([128, 128], F32)
        make_identity(nc, ident_f[:])
        ident_b = persist.tile([128, 128], BF16)
        make_identity(nc, ident_b[:])

        def emit_kv_half(c: int, h: int):
            kv_ps = psum_s.tile([128, QC], F32, name="kvps", tag="ps")
            for j in range(4):
                nc.tensor.matmul(
                    kv_ps[:],
                    wkv[:, j, :],
                    xb[:, j, ds(c * 1024 + h * QC, QC)],
                    start=(j == 0),
                    stop=(j == 3),
                )
            nc.scalar.activation(
                kvt[:, ds(c * 1024 + h * QC, QC)], kv_ps[:], AF.Identity,
                bias=bias_kv[:],
            )
            vp = psum_s.tile([128, 4, D], BF16, name="vp", tag="ps")
            kt0 = 8 * c + 4 * h
            for u in range(4):
                nc.tensor.transpose(
                    vp[:, u, :],
                    kvt[D : 2 * D, ts(kt0 + u, 128)],
                    ident_b[D : 2 * D, D : 2 * D],
                )
            # ACT (not DVE): vp shares the rotating PSUM pool with the score
            # tiles, and the DVE queue is 2 mask-TTs deep - reading vp out on
            # the scalar engine frees the slab ~1.5us sooner per kv slot.
            nc.scalar.copy(vaug[:, kt0 : kt0 + 4, 0:D], vp[:])

        def emit_q(qh: int):
            q_ps = psum_s.tile([D, QH], F32, name="qps", tag="ps")
            for h in range(QH // QC):
                for j in range(4):
                    nc.tensor.matmul(
                        q_ps[:, ts(h, QC)],
                        wq[:, j, :],
                        xb[:, j, ds(qh * QH + h * QC, QC)],
                        start=(j == 0),
                        stop=(j == 3),
                    )
            nc.scalar.activation(
                qt_t[0:D, ts(qh, QH)], q_ps[:], AF.Identity, bias=bias_q[:]
            )

        def emit_scores_exp(qh, kt, mk):
            st = psum_s.tile([128, QH], F32, name="st", tag="ps")
            for qc in range(QH // QC):
                nc.tensor.matmul(
                    st[:, ts(qc, QC)],
                    kvt[:, ts(kt, 128)],
                    qt_t[:, ds(qh * QH + qc * QC, QC)],
                    start=True,
                    stop=True,
                )
            sm = sm_pool.tile([128, QH], BF16, tag="sm")
            nc.vector.tensor_tensor(out=sm[:], in0=st[:], in1=mk, op=ALU.mult)
            pt = pt_pool.tile([128, QH], BF16, tag="pt")
            nc.scalar.activation(pt[:], sm[:], AF.Exp, scale=0.125)
            return pt

        def emit_pv(qh, kt, ot, pt, first, last):
            for qc in range(QH // QC):
                nc.tensor.matmul(
                    ot[:, ds(qh * QH + qc * QC, QC)],
                    vaug[:, kt, :],
                    pt[:, ts(qc, QC)],
                    start=first,
                    stop=last,
                )

        def emit_epilogue_half(ot, half):
            # one q-half: PSUM->SBUF copy, 8 PE transposes, strided
            # reciprocal of the denominator row, divide, store
            ots = epi.tile([D + 1, QH], F32, name=f"ots{half}", tag=f"ots{half}")
            nc.scalar.copy(ots[:], ot[:, ts(half, QH)])
            rcp = epi2.tile([128, 16], F32, tag="rcp")
            of = epi2.tile([128, 8, D], F32, tag=f"of{half}")
            oext = out_ext[:].rearrange("(hf qt p) d -> hf p qt d", hf=2, p=128)
            op8 = psum_s.tile([128, 8, 128], F32, name="op8", tag="ps")
            for i in range(8):
                nc.tensor.transpose(
                    op8[:, i, 0 : D + 1], ots[:, ts(i, 128)],
                    ident_f[0 : D + 1, 0 : D + 1],
                )
            nc.vector.reciprocal(
                rcp[:, ts(half, 8)], op8[:, :, D : D + 1].rearrange("p t o -> p (t o)")
            )
            for i in range(8):
                nc.vector.tensor_scalar(
                    of[:, i, :], op8[:, i, 0:D], rcp[:, 8 * half + i : 8 * half + i + 1],
                    None, op0=ALU.mult,
                )
            nc.sync.dma_start(oext[half], of[:])

        # ---------------- emission ----------------
        ot = psum_o.tile([D + 1, Q_LOC], F32, name="ot", tag="ot")
        emit_kv_half(0, 0)
        emit_kv_half(0, 1)
        emit_q(0)
        emit_q(1)
        # KV chunk c+1 interleaves during chunk c's k-tiles (2 halves over
        # 8 k-tiles; x column-chunks stream in well ahead of their deadline)
        # PV lags TWO k-tiles behind the scores so the in-order PE queue
        # never waits on a just-finished exp.
        pending = []  # [(kt, [pt_qh0, pt_qh1]), ...]
        for c in range(4):
            nxt = []
            if c + 1 < 4:
                nxt = [(c + 1, h) for h in range(2)]
            for i, kt in enumerate(range(8 * c, 8 * c + 8)):
                # kv-half BEFORE this kt's scores: its matmuls fill the PE's
                # TT-wait gap and its ACT bias-copy precedes the exps in the
                # scalar queue, freeing the shared PSUM slab sooner
                if i in (2, 5) and nxt:
                    emit_kv_half(*nxt.pop(0))
                pts = []
                for qh in range(2):
                    mk = mg[kt // 4][:, kt % 4, ts(qh, QH)]
                    pts.append(emit_scores_exp(qh, kt, mk))
                pending.append((kt, pts))
                if len(pending) > 3:
                    pkt, ppts = pending.pop(0)
                    for qh in range(2):
                        emit_pv(qh, pkt, ot, ppts[qh], pkt == 0, False)
        # flush: the qh0 half of the epilogue overlaps the final qh1 PVs
        for pkt, ppts in pending[:-1]:
            for qh in range(2):
                emit_pv(qh, pkt, ot, ppts[qh], False, False)
        k31, p31 = pending[-1]
        emit_pv(0, k31, ot, p31[0], False, True)
        emit_epilogue_half(ot, 0)
        emit_pv(1, k31, ot, p31[1], False, True)
        emit_epilogue_half(ot, 1)

    nc.compile()
    return nc


def _shard_inputs(input_embedding, mask, Wq, bq, Wk, bk, Wv, bv):
    import ml_dtypes

    input_embedding = np.asarray(input_embedding, dtype=np.float32)
    mask = np.asarray(mask, dtype=np.int32)

    def pack_w(w):
        return np.ascontiguousarray(
            np.asarray(w, np.float32).reshape(4, 128, -1).transpose(1, 0, 2)
        ).astype(ml_dtypes.bfloat16)

    wkv = np.concatenate([pack_w(Wk), pack_w(Wv)], axis=2)
    w = {
        "wkv": np.ascontiguousarray(wkv),
        "wq": pack_w(Wq),
        "bkv": np.ascontiguousarray(
            np.concatenate([np.asarray(bk, np.float32), np.asarray(bv, np.float32)]).reshape(-1, 1)
        ),
        "bq": np.ascontiguousarray(np.asarray(bq, np.float32).reshape(-1, 1)),
    }
    in_maps = []
    for c in range(N_CORES):
        b, qh = divmod(c, 2)
        # x^T [C, S] bf16, rolled so this core's q-slab is at [0:Q_LOC)
        x_c = np.roll(input_embedding[b].T, -Q_LOC * qh, axis=1).astype(
            ml_dtypes.bfloat16
        )
        # mask^T slab [S(k), Q_LOC(q)] rolled along k, packed so group g's
        # partition p holds k-rows {g*512 + t*128 + p} (8KB contiguous)
        m_c = np.roll(mask[Q_LOC * qh : Q_LOC * (qh + 1), :].T, -Q_LOC * qh, axis=0)
        m_p = (
            m_c.astype(np.int8)
            .reshape(N_MG, 4, 128, Q_LOC)
            .transpose(0, 2, 1, 3)
            .reshape(N_MG, 128, 4 * Q_LOC)
        )
        in_maps.append(
            {
                "xt": np.ascontiguousarray(x_c),
                "maskp": np.ascontiguousarray(m_p),
                **w,
            }
        )
    return in_maps


def _gather(results):
    out = np.empty((B, S, D), dtype=np.float32)
    for c in range(N_CORES):
        b, qh = divmod(c, 2)
        out[b, Q_LOC * qh : Q_LOC * (qh + 1), :] = results[c]["out"]
    return out


def kernel(input_embedding, mask, Wq, bq, Wk, bk, Wv, bv):
    nc = build_kernel()
    in_maps = _shard_inputs(input_embedding, mask, Wq, bq, Wk, bk, Wv, bv)
    res = run_bass_kernel_spmd(nc, in_maps, list(range(N_CORES)))
    return _gather(res.results)
